# revision 14
# baseline (speedup 1.0000x reference)
"""Trainium2 Bass kernel for nn_Block_62156766708387 (moe_routing).

Transformer block: x + attn(LN1(x)), then + top2-MoE(LN2(.)).

Execution plan (8 NeuronCores):
  Launch A  (data-parallel over batch, 1 batch element / core):
      attention -> x1 = x + attnout.  The host derives the MoE gating
      logits from x1, and the reference's min 2nd-vs-3rd logit gap is
      ~1e-5, so x1 must be fp32-accurate or top-2 routing flips vs the
      reference and blows the absmax error.  QKV / scores / out-proj
      matmuls therefore run as compensated fp16 pairs (a = hi + lo with
      hi = f16(a), lo = f16(a - hi); products hi*hi + hi*lo + lo*hi
      accumulate exactly in the fp32 PSUM -> ~1e-6 relative accuracy at
      fp16 speed), and softmax/AV run in true fp32.  Verified margin vs
      routing flips: max logit perturbation 9e-7 vs min half-gap 4.8e-6.
  Host:     LN2 + gate logits (fp64), top-2 routing, per-expert gather.
  Launch B  (expert-parallel, expert e on core e):
      fp16 FFN y = gelu(tok @ W1 + b1) @ W2 + b2 over CAP token slots.
  Host:     weighted scatter-add + residual.
"""

import numpy as np
import ml_dtypes

import concourse.bass as bass
import concourse.tile as tile
from concourse import bacc, mybir
from concourse import bass_utils
from concourse.bass import ts

F32 = mybir.dt.float32
F16 = mybir.dt.float16
BF16 = mybir.dt.bfloat16

B, T, D = 8, 1024, 1024
H = 4 * D
E = 8
NH, HD = 16, 64
EPS = 1e-5
N_CORES = 8
PT = T // 128    # 8   T tiles
PD = D // 128    # 8   D tiles
PH = H // 128    # 32  H tiles
CAP = 2176       # token slots per expert (max observed count 2158)
CHUNKS = [512, 512, 512, 512, 128]
assert sum(CHUNKS) == CAP

_CACHE = {}


# --------------------------------------------------------------------------
# Launch A: attention block (per-core = one batch element)
# --------------------------------------------------------------------------
def _build_attn(reps=1, cfg=None):
    cfg = cfg or {}
    nc = bacc.Bacc("TRN2", target_bir_lowering=False, debug=False,
                   num_devices=N_CORES)
    x_d = nc.dram_tensor("x", [T, D], F32, kind="ExternalInput")
    h1h_d = nc.dram_tensor("h1t_hi", [D, T], F16, kind="ExternalInput")
    h1l_d = nc.dram_tensor("h1t_lo", [D, T], F16, kind="ExternalInput")
    w_hi, w_lo = {}, {}
    for w in ("wq", "wk", "wv", "wp"):
        w_hi[w] = nc.dram_tensor(w + "_hi", [D, D], F16, kind="ExternalInput")
        w_lo[w] = nc.dram_tensor(w + "_lo", [D, D], F16, kind="ExternalInput")
    bq_d = nc.dram_tensor("bq8", [D], F32, kind="ExternalInput")   # bq/8
    bk_d = nc.dram_tensor("bk", [D], F32, kind="ExternalInput")
    bv_d = nc.dram_tensor("bv", [1, D], F16, kind="ExternalInput")
    bp_d = nc.dram_tensor("bp", [1, D], F16, kind="ExternalInput")
    msk_d = nc.dram_tensor("masks", [4, 128, 512], F32, kind="ExternalInput")
    idn_d = nc.dram_tensor("ident", [128, 128], F32, kind="ExternalInput")
    one_d = nc.dram_tensor("onesc", [1, 128], F16, kind="ExternalInput")
    x1_d = nc.dram_tensor("x1", [T, D], F32, kind="ExternalOutput")

    x_r = x_d.ap().rearrange("(a p) n -> p a n", p=128)       # [128, 8, 1024]
    h1h_r = h1h_d.ap().rearrange("(a p) t -> p a t", p=128)
    h1l_r = h1l_d.ap().rearrange("(a p) t -> p a t", p=128)
    x1_r = x1_d.ap().rearrange("(a p) n -> p a n", p=128)

    with tile.TileContext(nc) as tc:
        with (
            tc.tile_pool(name="consts", bufs=1) as consts,
            tc.tile_pool(name="small", bufs=8) as small,
            tc.tile_pool(name="qkv", bufs=1) as qkv,
        ):
            ident = consts.tile([128, 128], F32)
            nc.sync.dma_start(out=ident[:], in_=idn_d.ap())
            masks = consts.tile([128, 4, 512], F32)
            nc.sync.dma_start(out=masks[:], in_=msk_d.ap().rearrange("m p c -> p m c"))
            onesc = consts.tile([1, 128], F16)
            nc.sync.dma_start(out=onesc[:], in_=one_d.ap())
            bq_t = consts.tile([128, PD], F32)
            nc.sync.dma_start(out=bq_t[:], in_=bq_d.ap().rearrange("(a p) -> p a", p=128))
            bk_t = consts.tile([128, PD], F32)
            nc.sync.dma_start(out=bk_t[:], in_=bk_d.ap().rearrange("(a p) -> p a", p=128))
            bv_r = consts.tile([1, D], F16)
            nc.sync.dma_start(out=bv_r[:], in_=bv_d.ap())
            bp_r = consts.tile([1, D], F16)
            nc.sync.dma_start(out=bp_r[:], in_=bp_d.ap())

            qTh = qkv.tile([128, PD, T], F16)
            qTl = qkv.tile([128, PD, T], F16)
            kTh = qkv.tile([128, PD, T], F16)
            kTl = qkv.tile([128, PD, T], F16)
            vaug = qkv.tile([128, PT, NH, HD + 1], F32)
            yTh = qkv.tile([128, PD, T], F16)
            yTl = qkv.tile([128, PD, T], F16)
            nc.gpsimd.memset(vaug[:, :, :, HD:HD + 1], 1.0)

            for rep in range(reps):
                # ---------------- QKV projections (f16x2) ------------------
                with (
                    tc.tile_pool(name=f"h1p{rep}", bufs=1) as h1p,
                    tc.tile_pool(name=f"wpool{rep}", bufs=cfg.get("wpool", 4)) as wpool,
                    tc.tile_pool(name=f"qtmp{rep}", bufs=cfg.get("qtmp", 3)) as qtmp,
                    tc.tile_pool(name=f"psC{rep}", bufs=cfg.get("psC", 3),
                                 space=bass.MemorySpace.PSUM) as psC,
                ):
                    h1h = h1p.tile([128, PD, T], F16)
                    h1l = h1p.tile([128, PD, T], F16)
                    for a in range(PD):
                        nc.sync.dma_start(out=h1h[:, a, :], in_=h1h_r[:, a, :])
                        nc.sync.dma_start(out=h1l[:, a, :], in_=h1l_r[:, a, :])

                    def mm24(ps, wh, wl, hh, hl, n512, first, last):
                        """hi*hi + hi*lo + lo*hi accumulation over one k-chunk."""
                        nc.tensor.matmul(ps, wh, hh, start=first, stop=False)
                        nc.tensor.matmul(ps, wh, hl, start=False, stop=False)
                        nc.tensor.matmul(ps, wl, hh, start=False, stop=last)

                    for wname, dsth, dstl, b_t, scale in (
                        ("wq", qTh, qTl, bq_t, 0.125),
                        ("wk", kTh, kTl, bk_t, 1.0),
                    ):
                        wrh = w_hi[wname].ap().rearrange("(k p) n -> p k n", p=128)
                        wrl = w_lo[wname].ap().rearrange("(k p) n -> p k n", p=128)
                        for quad in range(4):
                            wth = wpool.tile([128, PD, 256], F16, tag="w")
                            wtl = wpool.tile([128, PD, 256], F16, tag="w")
                            for kk in range(PD):
                                nc.sync.dma_start(out=wth[:, kk, :],
                                                  in_=wrh[:, kk, ts(quad, 256)])
                                nc.sync.dma_start(out=wtl[:, kk, :],
                                                  in_=wrl[:, kk, ts(quad, 256)])
                            for jl in range(2):
                                j = 2 * quad + jl
                                for n in range(T // 512):
                                    ps = psC.tile([128, 512], F32)
                                    for kk in range(PD):
                                        mm24(ps[:], wth[:, kk, ts(jl, 128)],
                                             wtl[:, kk, ts(jl, 128)],
                                             h1h[:, kk, ts(n, 512)],
                                             h1l[:, kk, ts(n, 512)],
                                             n, kk == 0, kk == PD - 1)
                                    q32 = qtmp.tile([128, 512], F32, tag="q32")
                                    nc.scalar.activation(
                                        q32[:], ps[:],
                                        mybir.ActivationFunctionType.Identity,
                                        bias=b_t[:, j:j + 1], scale=scale)
                                    nc.scalar.copy(dsth[:, j, ts(n, 512)], q32[:])
                                    nc.vector.tensor_sub(
                                        dstl[:, j, ts(n, 512)], q32[:],
                                        dsth[:, j, ts(n, 512)])

                    # V token-major (fp32 result) with ones column per head
                    wrh = w_hi["wv"].ap().rearrange("(k p) n -> p k n", p=128)
                    wrl = w_lo["wv"].ap().rearrange("(k p) n -> p k n", p=128)
                    for n in range(D // 256):
                        wth = wpool.tile([128, PD, 256], F16, tag="w")
                        wtl = wpool.tile([128, PD, 256], F16, tag="w")
                        for kk in range(PD):
                            nc.sync.dma_start(out=wth[:, kk, :],
                                              in_=wrh[:, kk, ts(n, 256)])
                            nc.sync.dma_start(out=wtl[:, kk, :],
                                              in_=wrl[:, kk, ts(n, 256)])
                        for i in range(PT):
                            ps = psC.tile([128, 256], F32, tag="psv")
                            for kk in range(PD):
                                nc.tensor.matmul(ps[:], h1h[:, kk, ts(i, 128)],
                                                 wth[:, kk, :],
                                                 start=(kk == 0), stop=False)
                                nc.tensor.matmul(ps[:], h1l[:, kk, ts(i, 128)],
                                                 wth[:, kk, :],
                                                 start=False, stop=False)
                                nc.tensor.matmul(ps[:], h1h[:, kk, ts(i, 128)],
                                                 wtl[:, kk, :],
                                                 start=False, stop=False)
                            nc.tensor.matmul(ps[:], onesc[:, :],
                                             bv_r[:, ts(n, 256)],
                                             start=False, stop=True)
                            nc.scalar.copy(
                                vaug[:, i, 4 * n:4 * n + 4, 0:HD],
                                ps[:].rearrange("p (h c) -> p h c", h=4))

                # ------------- attention: scores f16x2, softmax/AV fp32 ----
                with (
                    tc.tile_pool(name=f"expool{rep}", bufs=cfg.get("expool", 12)) as expool,
                    tc.tile_pool(name=f"ytmp{rep}", bufs=cfg.get("ytmp", 4)) as ytmp,
                    tc.tile_pool(name=f"psS{rep}", bufs=cfg.get("psS", 3),
                                 space=bass.MemorySpace.PSUM) as psS,
                    tc.tile_pool(name=f"psY{rep}", bufs=cfg.get("psY", 4),
                                 space=bass.MemorySpace.PSUM) as psY,
                    tc.tile_pool(name=f"psT{rep}", bufs=cfg.get("psT", 1),
                                 space=bass.MemorySpace.PSUM) as psT,
                ):
                    for n in range(T // 512):
                        jmax = 4 * (n + 1)
                        for h in range(NH):
                            hp0 = (h % 2) * 64
                            hj = h // 2
                            hsl = slice(hp0, hp0 + 64)
                            blocks = []
                            for j in range(jmax):
                                ps = psS.tile([128, 512], F32)
                                nc.tensor.matmul(ps[:], kTh[hsl, hj, ts(j, 128)],
                                                 qTh[hsl, hj, ts(n, 512)],
                                                 start=True, stop=False)
                                nc.tensor.matmul(ps[:], kTh[hsl, hj, ts(j, 128)],
                                                 qTl[hsl, hj, ts(n, 512)],
                                                 start=False, stop=False)
                                nc.tensor.matmul(ps[:], kTl[hsl, hj, ts(j, 128)],
                                                 qTh[hsl, hj, ts(n, 512)],
                                                 start=False, stop=True)
                                es = expool.tile([128, 512], F32, tag="es")
                                nc.scalar.activation(
                                    es[:], ps[:],
                                    mybir.ActivationFunctionType.Exp)
                                r = j - 4 * n
                                if r >= 0:
                                    nc.vector.tensor_mul(es[:], es[:],
                                                         masks[:, r, :])
                                blocks.append(es)
                            for qt in range(4):
                                it = 4 * n + qt
                                psy = psY.tile([128, HD + 1], F32)
                                for j in range(it + 1):
                                    nc.tensor.matmul(
                                        psy[:], blocks[j][:, ts(qt, 128)],
                                        vaug[:, j, h, :],
                                        start=(j == 0), stop=(j == it))
                                rc = small.tile([128, 1], F32, tag="rc")
                                nc.vector.reciprocal(rc[:], psy[:, HD:HD + 1])
                                yt = ytmp.tile([128, HD], F32, tag="yt")
                                nc.scalar.mul(yt[:], psy[:, 0:HD], rc[:])
                                pst = psT.tile([64, 128], F32)
                                nc.tensor.transpose(pst[:], yt[:], ident[:])
                                nc.scalar.copy(yTh[hsl, hj, ts(it, 128)], pst[:])
                                nc.vector.tensor_sub(yTl[hsl, hj, ts(it, 128)],
                                                     pst[:],
                                                     yTh[hsl, hj, ts(it, 128)])

                # ---------------- output proj (f16x2) + residual -----------
                with (
                    tc.tile_pool(name=f"wpool2{rep}", bufs=4) as wpool2,
                    tc.tile_pool(name=f"xr{rep}", bufs=4) as xr,
                    tc.tile_pool(name=f"xo{rep}", bufs=4) as xo,
                    tc.tile_pool(name=f"psP{rep}", bufs=3,
                                 space=bass.MemorySpace.PSUM) as psP,
                ):
                    wrh = w_hi["wp"].ap().rearrange("(k p) n -> p k n", p=128)
                    wrl = w_lo["wp"].ap().rearrange("(k p) n -> p k n", p=128)
                    for n in range(D // 512):
                        wth = wpool2.tile([128, PD, 512], F16, tag="wp")
                        wtl = wpool2.tile([128, PD, 512], F16, tag="wp")
                        for kk in range(PD):
                            nc.sync.dma_start(out=wth[:, kk, :],
                                              in_=wrh[:, kk, ts(n, 512)])
                            nc.sync.dma_start(out=wtl[:, kk, :],
                                              in_=wrl[:, kk, ts(n, 512)])
                        for i in range(PT):
                            xt = xr.tile([128, 512], F32, tag="xt")
                            nc.sync.dma_start(out=xt[:], in_=x_r[:, i, ts(n, 512)])
                            ps = psP.tile([128, 512], F32)
                            for kk in range(PD):
                                nc.tensor.matmul(ps[:], yTh[:, kk, ts(i, 128)],
                                                 wth[:, kk, :],
                                                 start=(kk == 0), stop=False)
                                nc.tensor.matmul(ps[:], yTh[:, kk, ts(i, 128)],
                                                 wtl[:, kk, :],
                                                 start=False, stop=False)
                                nc.tensor.matmul(ps[:], yTl[:, kk, ts(i, 128)],
                                                 wth[:, kk, :],
                                                 start=False, stop=False)
                            nc.tensor.matmul(ps[:], onesc[:, :],
                                             bp_r[:, ts(n, 512)],
                                             start=False, stop=True)
                            x1t = xo.tile([128, 512], F32, tag="x1t")
                            nc.vector.tensor_add(x1t[:], ps[:], xt[:])
                            nc.sync.dma_start(out=x1_r[:, i, ts(n, 512)],
                                              in_=x1t[:])

    nc.compile()
    return nc


# --------------------------------------------------------------------------
# Launch B: expert FFN (per-core = one expert), fp16
# --------------------------------------------------------------------------
def _build_expert(reps=1):
    nc = bacc.Bacc("TRN2", target_bir_lowering=False, debug=False,
                   num_devices=N_CORES)
    tokt_d = nc.dram_tensor("tokt", [D, CAP], F16, kind="ExternalInput")
    w1_d = nc.dram_tensor("w1", [D, H], F16, kind="ExternalInput")
    w2_d = nc.dram_tensor("w2", [H, D], F16, kind="ExternalInput")
    b1_d = nc.dram_tensor("b1", [H], F32, kind="ExternalInput")
    b2_d = nc.dram_tensor("b2", [1, D], F16, kind="ExternalInput")
    one_d = nc.dram_tensor("onesc", [1, 128], F16, kind="ExternalInput")
    y_d = nc.dram_tensor("y", [CAP, D], F32, kind="ExternalOutput")

    tokt_r = tokt_d.ap().rearrange("(k p) c -> p k c", p=128)
    y_r = y_d.ap().rearrange("(a p) n -> p a n", p=128)

    with tile.TileContext(nc) as tc:
        with (
            tc.tile_pool(name="wpool", bufs=1) as wpool,
            tc.tile_pool(name="consts", bufs=1) as consts,
            tc.tile_pool(name="tokp", bufs=2) as tokp,
            tc.tile_pool(name="midp", bufs=1) as midp,
            tc.tile_pool(name="ysb", bufs=4) as ysbp,
            tc.tile_pool(name="psA", bufs=2, space=bass.MemorySpace.PSUM) as psA,
            tc.tile_pool(name="psB", bufs=2, space=bass.MemorySpace.PSUM) as psB,
        ):
            w1 = wpool.tile([128, PD, H], F16)
            w1r = w1_d.ap().rearrange("(k p) n -> p k n", p=128)
            for kk in range(PD):
                nc.sync.dma_start(out=w1[:, kk, :], in_=w1r[:, kk, :])
            w2 = wpool.tile([128, PH, D], F16)
            w2r = w2_d.ap().rearrange("(k p) n -> p k n", p=128)
            for kk in range(PH):
                nc.sync.dma_start(out=w2[:, kk, :], in_=w2r[:, kk, :])
            b1_t = consts.tile([128, PH], F32)
            nc.sync.dma_start(out=b1_t[:], in_=b1_d.ap().rearrange("(a p) -> p a", p=128))
            b2_r = consts.tile([1, D], F16)
            nc.sync.dma_start(out=b2_r[:], in_=b2_d.ap())
            onesc = consts.tile([1, 128], F16)
            nc.sync.dma_start(out=onesc[:], in_=one_d.ap())

            for rep in range(reps):
                for ci, cw in enumerate(CHUNKS):
                    c0 = 512 * ci
                    tokc = tokp.tile([128, PD, 512], F16, tag="tok")
                    for kk in range(PD):
                        nc.sync.dma_start(out=tokc[:, kk, :cw],
                                          in_=tokt_r[:, kk, c0:c0 + cw])
                    midc = midp.tile([128, PH, 512], F16, tag="mid")
                    for hj in range(PH):
                        ps = psA.tile([128, 512], F32)
                        for kk in range(PD):
                            nc.tensor.matmul(ps[:, :cw], w1[:, kk, ts(hj, 128)],
                                             tokc[:, kk, :cw],
                                             start=(kk == 0), stop=(kk == PD - 1))
                        nc.scalar.activation(midc[:, hj, :cw], ps[:, :cw],
                                             mybir.ActivationFunctionType.Gelu,
                                             bias=b1_t[:, hj:hj + 1])
                    for ti in range(cw // 128):
                        for nn in range(D // 512):
                            ps2 = psB.tile([128, 512], F32)
                            for hj in range(PH):
                                nc.tensor.matmul(ps2[:], midc[:, hj, ts(ti, 128)],
                                                 w2[:, hj, ts(nn, 512)],
                                                 start=(hj == 0), stop=False)
                            nc.tensor.matmul(ps2[:], onesc[:, :],
                                             b2_r[:, ts(nn, 512)],
                                             start=False, stop=True)
                            ysb = ysbp.tile([128, 512], F32, tag="y")
                            nc.scalar.copy(ysb[:], ps2[:])
                            nc.sync.dma_start(
                                out=y_r[:, 4 * ci + ti, ts(nn, 512)], in_=ysb[:])

    nc.compile()
    return nc


# --------------------------------------------------------------------------
# Host-side pieces
# --------------------------------------------------------------------------
def _layernorm64(x, g, b):
    x = x.astype(np.float64)
    mu = x.mean(axis=-1, keepdims=True)
    var = ((x - mu) ** 2).mean(axis=-1, keepdims=True)
    return ((x - mu) / np.sqrt(var + EPS)) * g + b


def _causal_masks():
    m = np.zeros((4, 128, 512), np.float32)
    p = np.arange(128)[:, None]
    c = np.arange(512)[None, :]
    for r in range(4):
        m[r] = (c - p >= r * 128).astype(np.float32)
    return m


def _split16(a):
    """fp32 -> (hi, lo) fp16 pair with a ~= hi + lo."""
    a = np.asarray(a, np.float32)
    hi = a.astype(np.float16)
    lo = (a - hi.astype(np.float32)).astype(np.float16)
    return hi, lo


def _gelu_exact64(x):
    from math import erf
    v = np.vectorize(erf)
    return 0.5 * x * (1.0 + v(x / np.sqrt(2.0)))


def _get(name, builder):
    if name not in _CACHE:
        _CACHE[name] = builder()
    return _CACHE[name]


def _attn_in_maps(inp):
    x = np.ascontiguousarray(inp["x"], np.float32)
    h1 = _layernorm64(x, inp["ln1_g"].astype(np.float64),
                      inp["ln1_b"].astype(np.float64)).astype(np.float32)
    masks = _causal_masks()
    ident = np.eye(128, dtype=np.float32)
    onesc = np.ones((1, 128), np.float16)
    wsplit = {}
    for nm, key in (("wq", "Wq"), ("wk", "Wk"), ("wv", "Wv"), ("wp", "Wp")):
        hi, lo = _split16(np.ascontiguousarray(inp[key], np.float32))
        wsplit[nm + "_hi"] = hi
        wsplit[nm + "_lo"] = lo
    bq = inp["bq"].astype(np.float32)
    in_maps = []
    for b in range(B):
        h1t = np.ascontiguousarray(h1[b].T)
        h1t_hi, h1t_lo = _split16(h1t)
        in_maps.append({
            "x": x[b], "h1t_hi": h1t_hi, "h1t_lo": h1t_lo,
            **wsplit,
            "bq8": bq / 8.0, "bk": inp["bk"].astype(np.float32),
            "bv": inp["bv"].astype(np.float16)[None, :],
            "bp": inp["bp"].astype(np.float16)[None, :],
            "masks": masks, "ident": ident, "onesc": onesc,
        })
    return in_maps


def kernel(**inputs):
    inp = {k: np.asarray(v) for k, v in inputs.items()}
    gate_W = inp["gate_W"].astype(np.float64)
    gate_b = inp["gate_b"].astype(np.float64)
    exp_W1 = inp["exp_W1"]
    exp_b1 = inp["exp_b1"]
    exp_W2 = inp["exp_W2"]
    exp_b2 = inp["exp_b2"]

    ncA = _get("attn", _build_attn)
    ncB = _get("expert", _build_expert)

    in_maps_a = _attn_in_maps(inp)
    res_a = bass_utils.run_bass_kernel_spmd(ncA, in_maps_a,
                                            core_ids=list(range(N_CORES)))
    x1 = np.stack([res_a.results[b]["x1"] for b in range(B)])   # [B, T, D] f32

    # ---- host routing ----
    h2_64 = _layernorm64(x1, inp["ln2_g"].astype(np.float64),
                         inp["ln2_b"].astype(np.float64))
    flat = h2_64.reshape(-1, D)                                  # [N, D] f64
    logits = flat @ gate_W + gate_b                              # [N, E] f64
    N = flat.shape[0]
    i1 = np.argmax(logits, axis=1)
    l1 = logits[np.arange(N), i1]
    lm = logits.copy()
    lm[np.arange(N), i1] = -np.inf
    i2 = np.argmax(lm, axis=1)
    l2 = lm[np.arange(N), i2]
    e2 = np.exp(l2 - l1)
    wt1 = (1.0 / (1.0 + e2)).astype(np.float32)
    wt2 = (e2 / (1.0 + e2)).astype(np.float32)

    h2_16 = flat.astype(np.float32).astype(np.float16)
    tok_lists, wgt_lists, ovf = [], [], []
    in_maps_b = []
    onesc16 = np.ones((1, 128), np.float16)
    for e in range(E):
        sel1 = np.nonzero(i1 == e)[0]
        sel2 = np.nonzero(i2 == e)[0]
        toks = np.concatenate([sel1, sel2])
        wgts = np.concatenate([wt1[sel1], wt2[sel2]])
        if toks.shape[0] > CAP:
            ovf.append((e, toks[CAP:], wgts[CAP:]))
            toks, wgts = toks[:CAP], wgts[:CAP]
        tok_lists.append(toks)
        wgt_lists.append(wgts)
        tokt = np.zeros((D, CAP), np.float16)
        tokt[:, :toks.shape[0]] = h2_16[toks].T
        in_maps_b.append({
            "tokt": tokt,
            "w1": exp_W1[e].astype(np.float16),
            "w2": exp_W2[e].astype(np.float16),
            "b1": exp_b1[e].astype(np.float32),
            "b2": exp_b2[e].astype(np.float16)[None, :],
            "onesc": onesc16,
        })
    res_b = bass_utils.run_bass_kernel_spmd(ncB, in_maps_b,
                                            core_ids=list(range(N_CORES)))

    # ---- combine ----
    moe = np.zeros((N, D), np.float32)
    for e in range(E):
        toks, wgts = tok_lists[e], wgt_lists[e]
        y = res_b.results[e]["y"][:toks.shape[0]]
        moe[toks] += wgts[:, None] * y
    for e, toks, wgts in ovf:
        t64 = flat[toks]
        mid = _gelu_exact64(t64 @ exp_W1[e].astype(np.float64)
                            + exp_b1[e].astype(np.float64))
        yv = mid @ exp_W2[e].astype(np.float64) + exp_b2[e].astype(np.float64)
        moe[toks] += wgts[:, None] * yv.astype(np.float32)

    out = x1.reshape(N, D) + moe
    return out.reshape(B, T, D).astype(np.float32)

